# revision 1
# baseline (speedup 1.0000x reference)
"""DeepCapsNet Trainium2 kernel: 8-core data-parallel (batch 512 -> 8x64).

Pipeline per core (b=64, two sub-batches of 32, conv chunks of 4):
  conv1 3x3/s2 (1->128) + BN + ReLU   [PE, K=9 row-strip packed]
  conv2 3x3/s2 (128->256) + BN + ReLU [PE, K=128, 9 accumulating matmuls]
  conv3 depthwise 9x9 (256)           [GPSIMD mult + DVE reduce]
  squash -> 3x attention-routing capsule layers [PE strip matmuls + DVE/ACT]
All matmuls in float32r (TF32-like: 1 cycle/row, ~1e-4 relative error).
"""
import sys

sys.path.insert(0, "/opt/trn_rl_repo")
import numpy as np
import concourse.bass as bass
import concourse.tile as tile
from concourse import bacc, mybir
from concourse.bass_utils import run_bass_kernel_spmd

F32 = mybir.dt.float32
F32R = mybir.dt.float32r
AX = mybir.AxisListType
ALU = mybir.AluOpType
ACTF = mybir.ActivationFunctionType

B = 64          # per-core batch
NSB = 2         # sub-batches
SB = B // NSB   # 32
BC = 4          # conv chunk batch
NCHUNK = B // BC  # 16 chunks (8 per sub-batch)
BN_EPS = 1e-5
ISCALE = 1.0 / 2.8284271247461903
# capsule layer dims: (n_h, d_h); n_l = 32, d_l = 8 for all
CAPS = [(32, 8), (32, 8), (10, 16)]


def _bn_prep(nc, pool, g, b_, m, v, nch, epsbn):
    """Load BN params as [nch,1] columns; return (inv, shift) [nch, w] tiles
    where w = nch // 128 columns ... actually laid out [128, nhalf]."""
    nh = nch // 128 if nch > 128 else 1
    P = 128 if nch >= 128 else nch
    gt = pool.tile([P, nh], F32, tag=f"bn_g{nch}")
    bt = pool.tile([P, nh], F32, tag=f"bn_b{nch}")
    mt = pool.tile([P, nh], F32, tag=f"bn_m{nch}")
    vt = pool.tile([P, nh], F32, tag=f"bn_v{nch}")
    for h in range(nh):
        nc.sync.dma_start(gt[:, h], g[128 * h:128 * h + P])
        nc.sync.dma_start(bt[:, h], b_[128 * h:128 * h + P])
        nc.sync.dma_start(mt[:, h], m[128 * h:128 * h + P])
        nc.sync.dma_start(vt[:, h], v[128 * h:128 * h + P])
    sq = pool.tile([P, nh], F32, tag=f"bn_sq{nch}")
    nc.scalar.activation(sq[:], vt[:], ACTF.Sqrt, bias=epsbn[0:P, :])
    r = pool.tile([P, nh], F32, tag=f"bn_r{nch}")
    nc.vector.reciprocal(r[:], sq[:])
    inv = pool.tile([P, nh], F32, tag=f"bn_inv{nch}")
    nc.vector.tensor_tensor(inv[:], gt[:], r[:], ALU.mult)
    ms = pool.tile([P, nh], F32, tag=f"bn_ms{nch}")
    nc.vector.tensor_tensor(ms[:], mt[:], inv[:], ALU.mult)
    shift = pool.tile([P, nh], F32, tag=f"bn_sh{nch}")
    nc.vector.tensor_tensor(shift[:], bt[:], ms[:], ALU.subtract)
    return inv, shift


def build(debug=False):
    nc = bacc.Bacc("TRN2", target_bir_lowering=False, debug=False, num_devices=8)

    # ---- DRAM I/O (per-core shard shapes) ----
    x = nc.dram_tensor("x", [B, 1, 39, 39], F32R, kind="ExternalInput").ap()
    c1w = nc.dram_tensor("conv1_w", [128, 1, 3, 3], F32, kind="ExternalInput").ap()
    c1b = nc.dram_tensor("conv1_b", [128], F32, kind="ExternalInput").ap()
    c2w = nc.dram_tensor("conv2_w", [256, 128, 3, 3], F32, kind="ExternalInput").ap()
    c2b = nc.dram_tensor("conv2_b", [256], F32, kind="ExternalInput").ap()
    c3w = nc.dram_tensor("conv3_w", [256, 1, 9, 9], F32, kind="ExternalInput").ap()
    c3b = nc.dram_tensor("conv3_b", [256], F32, kind="ExternalInput").ap()
    bn = {}
    for i, nch in ((1, 128), (2, 256)):
        for p in "gbmv":
            bn[f"{i}{p}"] = nc.dram_tensor(f"bn{i}_{p}", [nch], F32, kind="ExternalInput").ap()
    Ws = [nc.dram_tensor(f"W{i+1}", [32, CAPS[i][0], 8, CAPS[i][1]], F32R,
                         kind="ExternalInput").ap() for i in range(3)]
    out = nc.dram_tensor("out", [B, 10, 16], F32, kind="ExternalOutput").ap()
    dbg = {}
    if debug:
        dbg["h1"] = nc.dram_tensor("dbg_h1", [128, B * 361], F32, kind="ExternalOutput").ap()
        dbg["h2"] = nc.dram_tensor("dbg_h2", [256, B * 81], F32, kind="ExternalOutput").ap()
        dbg["h3"] = nc.dram_tensor("dbg_h3", [256, B], F32, kind="ExternalOutput").ap()
        dbg["u1"] = nc.dram_tensor("dbg_u1", [B, 256], F32, kind="ExternalOutput").ap()
        dbg["u2"] = nc.dram_tensor("dbg_u2", [B, 256], F32, kind="ExternalOutput").ap()
        dbg["uT2"] = nc.dram_tensor("dbg_uT2", [2, 128, B], F32, kind="ExternalOutput").ap()
        dbg["S2"] = nc.dram_tensor("dbg_S2", [SB, 256], F32, kind="ExternalOutput").ap()
        dbg["A2"] = nc.dram_tensor("dbg_A2", [128, 8, 32], F32, kind="ExternalOutput").ap()
        dbg["C2"] = nc.dram_tensor("dbg_C2", [128, 8, 32], F32, kind="ExternalOutput").ap()
        dbg["Uh2d"] = nc.dram_tensor("dbg_Uh2d", [SB, 256], F32, kind="ExternalOutput").ap()

    with tile.TileContext(nc) as tc:
        import contextlib
        ctx = contextlib.ExitStack()
        with ctx:
            wpool = ctx.enter_context(tc.tile_pool(name="wpool", bufs=1))
            io = ctx.enter_context(tc.tile_pool(name="io", bufs=1))
            rep = ctx.enter_context(tc.tile_pool(name="rep", bufs=2))
            h1p = ctx.enter_context(tc.tile_pool(name="h1p", bufs=5))
            h2p = ctx.enter_context(tc.tile_pool(name="h2p", bufs=3))
            capsp = ctx.enter_context(tc.tile_pool(name="capsp", bufs=1))
            ps1 = ctx.enter_context(tc.tile_pool(name="ps1", bufs=3, space="PSUM"))
            ps2 = ctx.enter_context(tc.tile_pool(name="ps2", bufs=3, space="PSUM"))
            psc = ctx.enter_context(tc.tile_pool(name="psc", bufs=2, space="PSUM"))

            # DMA issue spread across engine queues (a single queue serializes
            # at ~600ns+size per transfer)
            _conv_engines = [nc.sync, nc.scalar]
            _dma_ctr = [0]

            def dma_rr(out_ap, in_ap):
                e = _conv_engines[_dma_ctr[0] % len(_conv_engines)]
                _dma_ctr[0] += 1
                e.dma_start(out_ap, in_ap)

            def dma_misc(out_ap, in_ap):
                nc.gpsimd.dma_start(out_ap, in_ap)

            # ================= weight prep =================
            epsbn = wpool.tile([128, 1], F32, tag="epsbn")
            nc.gpsimd.memset(epsbn[:], BN_EPS)
            epsn = wpool.tile([128, 1], F32, tag="epsn")
            nc.gpsimd.memset(epsn[:], 1e-30)
            inv1, shift1 = _bn_prep(nc, wpool, bn["1g"], bn["1b"], bn["1m"], bn["1v"], 128, epsbn)
            inv2, shift2 = _bn_prep(nc, wpool, bn["2g"], bn["2b"], bn["2m"], bn["2v"], 256, epsbn)
            # conv bias add: conv1_b/conv2_b are inputs; BN folds them via shift:
            # BN(conv + cb) = conv*inv + (cb*inv + shift). Fold cb into shift.
            c1bt = wpool.tile([128, 1], F32, tag="c1bt")
            nc.sync.dma_start(c1bt[:, 0], c1b[:])
            t_ = wpool.tile([128, 1], F32, tag="c1bs")
            nc.vector.tensor_tensor(t_[:], c1bt[:], inv1[:], ALU.mult)
            nc.vector.tensor_tensor(shift1[:], shift1[:], t_[:], ALU.add)
            c2bt = wpool.tile([128, 2], F32, tag="c2bt")
            for h in range(2):
                nc.sync.dma_start(c2bt[:, h], c2b[128 * h:128 * (h + 1)])
            t2_ = wpool.tile([128, 2], F32, tag="c2bs")
            nc.vector.tensor_tensor(t2_[:], c2bt[:], inv2[:], ALU.mult)
            nc.vector.tensor_tensor(shift2[:], shift2[:], t2_[:], ALU.add)
            # conv3 bias: h3 = depthwise(h2) + c3b -> fold into conv3 reduce output later
            c3bt = wpool.tile([128, 2], F32, tag="c3bt")
            for h in range(2):
                nc.sync.dma_start(c3bt[:, h], c3b[128 * h:128 * (h + 1)])

            # identity for PE transposes
            ident = wpool.tile([128, 128], F32, tag="ident")
            from concourse.masks import make_identity
            make_identity(nc, ident[:])

            # conv1 weights: natural [128, 9] -> PE transpose -> [9, 128],
            # replicate to 4 row strips -> w1T [128, 128] (strips at 32g, 9 rows used)
            w1nat = wpool.tile([128, 9], F32, tag="w1nat")
            nc.sync.dma_start(w1nat[:], c1w.rearrange("co i kh kw -> co (i kh kw)"))
            w1ps = psc.tile([9, 128], F32, tag="psct")
            nc.tensor.transpose(w1ps[:], w1nat[:], ident[:])
            w1T = wpool.tile([9, 128], F32R, tag="w1T")
            nc.vector.tensor_copy(w1T[:, :], w1ps[:, :])
            # conv2 weights: natural [2][128, 1152]; per (p, half) transpose ->
            # w2T [128, 18*128] slices [ci, co] (f32r)
            w2T = wpool.tile([128, 18, 128], F32R, tag="w2T")
            for h in range(2):
                w2nat = wpool.tile([128, 1152], F32, tag="w2nat", name="w2nat")
                nc.sync.dma_start(
                    w2nat[:], c2w.rearrange("co ci kh kw -> co (ci kh kw)")[128 * h:128 * (h + 1), :])
                for p in range(9):
                    wps = psc.tile([128, 128], F32, tag="psct")
                    nc.tensor.transpose(
                        wps[:], w2nat[:, p::9], ident[:])
                    nc.vector.tensor_copy(w2T[:, 2 * p + h, :], wps[:])
            # conv3 weights [2][128, 81]
            w3t = [wpool.tile([128, 81], F32, tag=f"w3_{h}", name=f"w3_{h}") for h in range(2)]
            for h in range(2):
                nc.sync.dma_start(w3t[h][:], c3w.rearrange("c i kh kw -> c (i kh kw)")[128 * h:128 * (h + 1), :])

            # x master [64, 39, 41] f32r, width padded to 41 (zeros) so conv1
            # can stream an even 20-wide ow dim (fp32r needs even innermost)
            xmr = io.tile([B, 1521], F32R, tag="xmr")
            nc.sync.dma_start(xmr[:], x.rearrange("b i h w -> b (i h w)"))
            xm = io.tile([B, 39, 41], F32R, tag="xm")
            nc.gpsimd.memset(xm[:, :, 39:41].bitcast(F32), 0.0)
            nc.vector.tensor_copy(xm[:, :, 0:39], xmr[:].bitcast(F32).rearrange("b (h w) -> b h w", h=39))
            xv = xm[:]

            # h3 accumulators (full core) [2][128, 64]
            h3t = [capsp.tile([128, B], F32, tag=f"h3_{h}", name=f"h3_{h}") for h in range(2)]

            # per-sub-batch capsule state
            u_out_sb = []
            W_bd, W_dn = None, None

            for sb in range(NSB):
                # ======== convs, chunked ========
                # chunk-groups of 4: conv1 for all 4 chunks, then conv2 with
                # one weight load per (p, half) shared across the 4 chunks
                GRP = 4
                for cg in range(NCHUNK // NSB // GRP):
                    h1cs = []
                    for ci in range(GRP):
                        c = sb * (NCHUNK // NSB) + cg * GRP + ci
                        Rt = rep.tile([9, BC, 37, 39], F32R, tag="R")
                        for p in range(9):
                            kh, kw = p // 3, p % 3
                            dma_rr(
                                Rt[p:p + 1, :, :, :],
                                xv[BC * c:BC * (c + 1), kh:kh + 37, kw:kw + 39],
                            )
                        h1c = h1p.tile([128, BC, 19, 20], F32R, tag="h1c")
                        for bb in range(BC):
                            pst = ps1.tile([128, 380], F32, tag="ps1t")
                            nc.tensor.matmul(
                                pst[:],
                                w1T[0:9, :],
                                Rt[:, bb, 0:37:2, 0:39:2],
                                start=True, stop=True,
                            )
                            nc.scalar.activation(
                                h1c[:, bb, :, :], pst[:].rearrange("p (oh ow) -> p oh ow", oh=19),
                                ACTF.Relu, bias=shift1[:, 0:1], scale=inv1[:, 0:1])
                        h1cs.append(h1c)
                        if debug:
                            nc.sync.dma_start(
                                dbg["h1"].rearrange("p (b oh ow) -> p b oh ow", b=B, oh=19)[:, BC * c:BC * (c + 1), :, :],
                                h1c[:, :, :, 0:19].bitcast(F32))

                    for h in range(2):
                        ps2ts = [ps2.tile([128, BC * 81], F32, tag="ps2t", name=f"ps2t{ci}")
                                 for ci in range(GRP)]
                        for p in range(9):
                            kh, kw = p // 3, p % 3
                            for ci in range(GRP):
                                nc.tensor.matmul(
                                    ps2ts[ci][:],
                                    w2T[:, 2 * p + h, :],
                                    h1cs[ci][:, :, kh:kh + 17:2, kw:kw + 17:2]
                                    .rearrange("p b oh ow -> p oh ow b"),
                                    start=(p == 0), stop=(p == 8),
                                )
                        for ci in range(GRP):
                            c = sb * (NCHUNK // NSB) + cg * GRP + ci
                            h2c = h2p.tile([128, BC, 81], F32, tag="h2c")
                            nc.scalar.activation(
                                h2c[:].rearrange("p b f -> p f b"),
                                ps2ts[ci][:], ACTF.Relu,
                                bias=shift2[:, h:h + 1], scale=inv2[:, h:h + 1])
                            t3 = h2p.tile([128, BC, 81], F32, tag="t3")
                            nc.gpsimd.tensor_tensor(
                                t3[:], h2c[:],
                                w3t[h][:, None, :].broadcast_to([128, BC, 81]), ALU.mult)
                            nc.vector.tensor_reduce(
                                h3t[h][:, BC * c:BC * (c + 1)], t3[:], axis=AX.X, op=ALU.add)
                            if debug:
                                nc.sync.dma_start(
                                    dbg["h2"].rearrange("(hh p) (b f) -> hh p b f", hh=2, b=B)[h, :, BC * c:BC * (c + 1), :],
                                    h2c[:])

                if sb == 0:
                # capsule weights: block-diag (strip matmul) + dense (for S)
                # W_bd[L][t] [128, 4*KL] ; W_dn[L][t] [128, KL]
                    W_bd, W_dn = [], []
                for L, (NH, DH) in enumerate(CAPS):
                    KL = NH * DH
                    # stage natural [32 (i), NH*8*DH], reorder free to (j, k, l)
                    Wst = wpool.tile([32, NH * 8 * DH], F32R, tag="Wst", name="Wst")
                    nc.sync.dma_start(Wst[:], Ws[L].rearrange("i k j l -> i (k j l)"))
                    Wst2 = wpool.tile([32, 8, NH, DH], F32R, tag="Wst2", name="Wst2")
                    nc.vector.tensor_copy(
                        Wst2[:],
                        Wst[:].bitcast(F32).rearrange("i (k j l) -> i j k l", k=NH, j=8))
                    bd_t, dn_t = [], []
                    for t in range(2):
                        bd = wpool.tile([128, 4, KL], F32R, tag=f"Wbd{L}{t}", name=f"Wbd{L}{t}")
                        nc.vector.memset(bd[:].bitcast(F32), 0.0)
                        dn = wpool.tile([128, KL], F32R, tag=f"Wdn{L}{t}", name=f"Wdn{L}{t}")
                        # dense: one DMA; partition (r, il, j) <- i=16t+4r+il, j
                        nc.sync.dma_start(
                            dn[:],
                            Wst2[16 * t:16 * (t + 1), :, :, :]
                            .rearrange("p j k l -> p j (k l)"))
                        # block-diag: per (il, r) DMAs (contiguous partition ranges;
                        # split-partition APs lower incorrectly in DMA)
                        for il in range(4):
                            for r in range(4):
                                ii = 16 * t + 4 * r + il
                                nc.sync.dma_start(
                                    bd[32 * r + 8 * il:32 * r + 8 * il + 8, il, :],
                                    Wst2[ii:ii + 1, :, :, :]
                                    .rearrange("p j k l -> p j (k l)"))
                        bd_t.append(bd)
                        dn_t.append(dn)
                    W_bd.append(bd_t)
                    W_dn.append(dn_t)


                # add conv3 bias: h3 += c3b (per partition)
                h3s = [capsp.tile([128, SB], F32, tag=f"h3s{h}", name=f"h3s{h}") for h in range(2)]
                for h in range(2):
                    nc.vector.tensor_tensor(
                        h3s[h][:], h3t[h][:, SB * sb:SB * (sb + 1)],
                        c3bt[:, h:h + 1].broadcast_to([128, SB]), ALU.add)
                if debug:
                    for h in range(2):
                        nc.sync.dma_start(dbg["h3"][128 * h:128 * (h + 1), SB * sb:SB * (sb + 1)], h3s[h][:])

                # ======== squash(u0) factors ========
                # h3T [SB, 256] via 8 stream transposes
                h3T = capsp.tile([SB, 256], F32, tag="h3T")
                for t in range(2):
                    for r in range(4):
                        nc.vector.transpose(
                            h3T[:, 32 * (4 * t + r):32 * (4 * t + r) + 32],
                            h3s[t][32 * r:32 * r + 32, :])
                sq = capsp.tile([SB, 256], F32, tag="sq0")
                nc.vector.tensor_tensor(sq[:], h3T[:], h3T[:], ALU.mult)
                ss = capsp.tile([SB, 32], F32, tag="ss0")
                nc.vector.tensor_reduce(ss[:], sq[:].rearrange("b (i j) -> b i j", i=32), axis=AX.X, op=ALU.add)
                nrm = capsp.tile([SB, 32], F32, tag="n0")
                nc.scalar.activation(nrm[:], ss[:], ACTF.Sqrt, bias=epsn[0:SB, :])
                ex = capsp.tile([SB, 32], F32, tag="e0")
                nc.scalar.activation(ex[:], nrm[:], ACTF.Exp, scale=-1.0)
                f_ = capsp.tile([SB, 32], F32, tag="f0")
                nc.vector.tensor_scalar(f_[:], ex[:], -1.0, 1.0, ALU.mult, ALU.add)
                rn = capsp.tile([SB, 32], F32, tag="rn0")
                nc.vector.reciprocal(rn[:], nrm[:])
                gf = capsp.tile([SB, 32], F32, tag="g0")
                nc.vector.tensor_tensor(gf[:], f_[:], rn[:], ALU.mult)
                # gT_rep[t] [128, SB]: partition 32r+8il+j <- g[b, 16t+4r+il]
                # built by stream-transposing j-replicated views of gf
                gT_rep = [capsp.tile([128, SB], F32, tag=f"gTr{t}", name=f"gTr{t}") for t in range(2)]
                G2 = capsp.tile([SB, 4, 8], F32, tag="G2")
                for t in range(2):
                    for r in range(4):
                        nc.vector.tensor_copy(
                            G2[:],
                            gf[:, 16 * t + 4 * r:16 * t + 4 * r + 4, None]
                            .broadcast_to([SB, 4, 8]))
                        nc.vector.transpose(
                            gT_rep[t][32 * r:32 * r + 32, :],
                            G2[:].rearrange("b il j -> b (il j)"))
                # uT for layer 1: dense [128,(b)] for S-matmul, plus block-diag
                # stationary [128,(cs,b)] for the U_hat accumulating matmuls
                # (fp32r psum must start at partition 0, so col-slot packing is
                # done via block-diagonal stationary operands instead)
                uT = [capsp.tile([128, SB], F32R, tag=f"uT{t}", name=f"uT{t}") for t in range(2)]
                uB = [capsp.tile([128, 128], F32R, tag=f"uB{t}", name=f"uB{t}") for t in range(2)]
                for t in range(2):
                    nc.vector.tensor_tensor(uT[t][:], h3s[t][:], gT_rep[t][:], ALU.mult)
                    nc.gpsimd.memset(uB[t][:].bitcast(F32), 0.0)
                    for r in range(4):
                        nc.vector.tensor_tensor(
                            uB[t][32 * r:32 * r + 32, 32 * r:32 * r + 32],
                            h3s[t][32 * r:32 * r + 32, :],
                            gT_rep[t][32 * r:32 * r + 32, :], ALU.mult)

                # ======== capsule layers ========
                for L, (NH, DH) in enumerate(CAPS):
                    KL = NH * DH
                    # S = sum_i U_hat: [SB, KL]
                    Sps = psc.tile([SB, KL], F32, tag="psct")
                    for t in range(2):
                        nc.tensor.matmul(Sps[:], uT[t][:], W_dn[L][t][:],
                                         start=(t == 0), stop=(t == 1))
                    Srep = capsp.tile([128, KL], F32, tag="Srep")
                    nc.vector.tensor_copy(Srep[0:SB, :], Sps[:])
                    for r in range(1, 4):
                        dma_misc(Srep[32 * r:32 * (r + 1), :], Srep[0:SB, :])
                    rS = capsp.tile([SB, KL], F32, tag="rS")
                    nc.vector.reciprocal(rS[:], Srep[0:SB, :])
                    if debug and L == 1 and sb == 0:
                        nc.sync.dma_start(dbg["S2"][:], Srep[0:SB, :])
                    # U_hat matmuls: 4 accumulating strip matmuls with a
                    # block-diagonal stationary -> full [128=(cs,b), 2*KL] psum
                    T = capsp.tile([128, 2, 4, KL], F32, tag="T")
                    for t in range(2):
                        for hh in range(2):
                            pst = psc.tile([128, 2 * KL], F32, tag="psct")
                            nc.tensor.matmul(
                                pst[:],
                                uB[t][:, :],
                                W_bd[L][t][:, 2 * hh:2 * hh + 2, :]
                                .rearrange("p i kl -> p (i kl)"),
                                start=True, stop=True,
                            )
                            nc.vector.tensor_tensor(
                                T[:, t, 2 * hh:2 * hh + 2, :],
                                pst[:].rearrange("p (i kl) -> p i kl", i=2),
                                Srep[:, None, :].broadcast_to([128, 2, KL]),
                                ALU.mult)
                    # A_sum = reduce_l(T) : [128, 8, NH]
                    A = capsp.tile([128, 8, NH], F32, tag="A")
                    nc.vector.tensor_reduce(
                        A[:], T[:].rearrange("p t i (k l) -> p (t i) k l", k=NH),
                        axis=AX.X, op=ALU.add)
                    # softmax over k (NH)
                    Mx = capsp.tile([128, 8], F32, tag="Mx")
                    nc.vector.tensor_reduce(Mx[:], A[:], axis=AX.X, op=ALU.max)
                    Ms = capsp.tile([128, 8], F32, tag="Ms")
                    nc.scalar.activation(Ms[:], Mx[:], ACTF.Copy, scale=ISCALE)
                    E = capsp.tile([128, 8, NH], F32, tag="E")
                    nc.vector.scalar_tensor_tensor(
                        E[:], A[:], ISCALE,
                        Ms[:, :, None].broadcast_to([128, 8, NH]),
                        ALU.mult, ALU.subtract)
                    nc.scalar.activation(E[:], E[:], ACTF.Exp)
                    Z = capsp.tile([128, 8], F32, tag="Z")
                    nc.vector.tensor_reduce(Z[:], E[:], axis=AX.X, op=ALU.add)
                    rZ = capsp.tile([128, 8], F32, tag="rZ")
                    nc.vector.reciprocal(rZ[:], Z[:])
                    C = capsp.tile([128, 8, NH], F32, tag="C")
                    nc.vector.tensor_tensor(
                        C[:], E[:], rZ[:, :, None].broadcast_to([128, 8, NH]), ALU.mult)
                    if debug and L == 1 and sb == 0:
                        nc.sync.dma_start(dbg["A2"][:], A[:])
                        nc.sync.dma_start(dbg["C2"][:], C[:])
                    # T2 = T * C (bcast l), in place
                    Tv = T[:].rearrange("p t i kl -> p (t i) kl").rearrange(
                        "p q (k l) -> p q k l", k=NH)
                    nc.vector.tensor_tensor(
                        Tv[:, 0:5], Tv[:, 0:5],
                        C[:, 0:5, :, None].broadcast_to([128, 5, NH, DH]), ALU.mult)
                    nc.gpsimd.tensor_tensor(
                        Tv[:, 5:8], Tv[:, 5:8],
                        C[:, 5:8, :, None].broadcast_to([128, 3, NH, DH]), ALU.mult)
                    # U_h'' = sum over q (strided innermost): [128, KL]
                    Uh2 = capsp.tile([128, KL], F32, tag="Uh2")
                    nc.vector.tensor_reduce(
                        Uh2[:].rearrange("p (k l) -> p k l", k=NH),
                        Tv.rearrange("p q k l -> p k l q"), axis=AX.X, op=ALU.add)
                    # fold cs strips: Uh' = sum_r Uh2[32r:32r+32]
                    scr = capsp.tile([SB, 3, KL], F32, tag="scr")
                    for r in range(1, 4):
                        dma_misc(scr[:, r - 1, :], Uh2[32 * r:32 * (r + 1), :])
                    Uh = capsp.tile([SB, KL], F32, tag="Uh")
                    nc.vector.tensor_tensor(Uh[:], Uh2[0:SB, :], scr[:, 0, :], ALU.add)
                    nc.vector.tensor_tensor(Uh[:], Uh[:], scr[:, 1, :], ALU.add)
                    nc.vector.tensor_tensor(Uh[:], Uh[:], scr[:, 2, :], ALU.add)
                    # U_h = Uh / S
                    nc.vector.tensor_tensor(Uh[:], Uh[:], rS[:], ALU.mult)
                    if debug and L == 1 and sb == 0:
                        nc.sync.dma_start(dbg["Uh2d"][:], Uh[:])
                    # squash
                    sq2 = capsp.tile([SB, KL], F32, tag="sq2")
                    nc.vector.tensor_tensor(sq2[:], Uh[:], Uh[:], ALU.mult)
                    ss2 = capsp.tile([SB, NH], F32, tag="ss2")
                    nc.vector.tensor_reduce(
                        ss2[:], sq2[:].rearrange("b (k l) -> b k l", k=NH), axis=AX.X, op=ALU.add)
                    n2 = capsp.tile([SB, NH], F32, tag="n2")
                    nc.scalar.activation(n2[:], ss2[:], ACTF.Sqrt, bias=epsn[0:SB, :])
                    e2 = capsp.tile([SB, NH], F32, tag="e2")
                    nc.scalar.activation(e2[:], n2[:], ACTF.Exp, scale=-1.0)
                    f2 = capsp.tile([SB, NH], F32, tag="f2")
                    nc.vector.tensor_scalar(f2[:], e2[:], -1.0, 1.0, ALU.mult, ALU.add)
                    rn2 = capsp.tile([SB, NH], F32, tag="rn2")
                    nc.vector.reciprocal(rn2[:], n2[:])
                    g2 = capsp.tile([SB, NH], F32, tag="g2")
                    nc.vector.tensor_tensor(g2[:], f2[:], rn2[:], ALU.mult)
                    if L < 2:
                        un = capsp.tile([SB, KL], F32, tag="un")
                        nc.vector.tensor_tensor(
                            un[:].rearrange("b (k l) -> b k l", k=NH),
                            Uh[:].rearrange("b (k l) -> b k l", k=NH),
                            g2[:, :, None].broadcast_to([SB, NH, DH]), ALU.mult)
                        if debug and L == 0:
                            nc.sync.dma_start(dbg["u1"][SB * sb:SB * (sb + 1), :], un[:])
                        if debug and L == 1:
                            nc.sync.dma_start(dbg["u2"][SB * sb:SB * (sb + 1), :], un[:])
                        # transposes for next layer (StreamTranspose is f32-only),
                        # then rounding copies into the f32r matmul operands
                        uTf = capsp.tile([128, SB], F32, tag="uTf")
                        uBf = capsp.tile([128, 128], F32, tag="uBf")
                        nc.gpsimd.memset(uBf[:], 0.0)
                        uT = [capsp.tile([128, SB], F32R, tag=f"uTn{t}", name=f"uTn{t}") for t in range(2)]
                        uB = [capsp.tile([128, 128], F32R, tag=f"uBn{t}", name=f"uBn{t}") for t in range(2)]
                        for t in range(2):
                            for r in range(4):
                                cblk = 4 * t + r
                                nc.vector.transpose(
                                    uTf[32 * r:32 * r + 32, :],
                                    un[:, 32 * cblk:32 * cblk + 32])
                                nc.vector.transpose(
                                    uBf[32 * r:32 * r + 32, 32 * r:32 * r + 32],
                                    un[:, 32 * cblk:32 * cblk + 32])
                            nc.vector.tensor_copy(uT[t][:], uTf[:])
                            nc.vector.tensor_copy(uB[t][:], uBf[:])
                        if debug and L == 0:
                            for t in range(2):
                                nc.sync.dma_start(dbg["uT2"][t, :, SB * sb:SB * (sb + 1)], uT[t][:].bitcast(F32))
                    else:
                        un = capsp.tile([SB, KL], F32, tag="unf")
                        nc.vector.tensor_tensor(
                            un[:].rearrange("b (k l) -> b k l", k=NH),
                            Uh[:].rearrange("b (k l) -> b k l", k=NH),
                            g2[:, :, None].broadcast_to([SB, NH, DH]), ALU.mult)
                        nc.sync.dma_start(
                            out[SB * sb:SB * (sb + 1), :, :],
                            un[:].rearrange("b (k l) -> b k l", k=NH))
                u_out_sb.append(None)

    nc.compile()
    return nc


_NC_CACHE = {}


def _get_nc(debug=False):
    key = debug
    if key not in _NC_CACHE:
        _NC_CACHE[key] = build(debug)
    return _NC_CACHE[key]


def kernel(**inputs):
    nc = _get_nc(False)
    x = np.ascontiguousarray(inputs["x"], dtype=np.float32)
    names = ["conv1_w", "conv1_b", "conv2_w", "conv2_b", "conv3_w", "conv3_b",
             "W1", "W2", "W3"]
    base = {n: np.ascontiguousarray(inputs[n], dtype=np.float32) for n in names}
    for i in (1, 2):
        for p in "gbmv":
            base[f"bn{i}_{p}"] = np.ascontiguousarray(inputs[f"bn{i}_{p}"], dtype=np.float32)
    in_maps = []
    for c in range(8):
        m = dict(base)
        m["x"] = x[B * c:B * (c + 1)]
        in_maps.append(m)
    res = run_bass_kernel_spmd(nc, in_maps, core_ids=list(range(8)))
    return np.concatenate([res.results[i]["out"] for i in range(8)], axis=0)



# revision 10
# speedup vs baseline: 1.3827x; 1.3827x over previous
"""DeepCapsNet Trainium2 kernel: 8-core data-parallel (batch 512 -> 8x64).

v1 redesign vs baseline (487us):
  - conv1 tap-replication via 9 strided SBUF->SBUF DMAs per 16-image block
    (was 144 small transposing DMAs); bf16 conv1/conv2 operands.
  - capsule block-diag weight DMA storm (96) replaced by il-masked dense
    moving operands (elementwise multiply with an affine-select mask).
  - Srep via 4x-replicated stationary matmul (no partition-replication DMAs);
    cs-strip fold via constant-matmul accumulation into PSUM (no fold DMAs).
  - sqrt -> exp(0.5*ln) so every ACT func lives in one activation table.
  - both 32-image caps chains independently buffered -> overlap each other
    and the other sub-batch's convs.
"""
import sys

sys.path.insert(0, "/opt/trn_rl_repo")
import numpy as np
import concourse.bass as bass
import concourse.tile as tile
from concourse import bacc, mybir
from concourse.bass_utils import run_bass_kernel_spmd
from concourse.masks import make_identity

F32 = mybir.dt.float32
F32R = mybir.dt.float32r
BF16 = mybir.dt.bfloat16
AX = mybir.AxisListType
ALU = mybir.AluOpType
ACTF = mybir.ActivationFunctionType

B = 64          # per-core batch
NSB = 2         # sub-batches (caps chains)
SB = B // NSB   # 32
BC = 4          # conv2/3 chunk batch
RBLK = 16       # images per conv1 Rt block
BN_EPS = 1e-5
ISCALE = 1.0 / 2.8284271247461903
CAPS = [(32, 8), (32, 8), (10, 16)]  # (n_h, d_h); n_l=32, d_l=8 throughout


def build():
    nc = bacc.Bacc("TRN2", target_bir_lowering=False, debug=False, num_devices=8)

    x = nc.dram_tensor("x", [B, 1, 39, 39], F32, kind="ExternalInput").ap()
    c1w = nc.dram_tensor("conv1_w", [128, 1, 3, 3], F32, kind="ExternalInput").ap()
    c1b = nc.dram_tensor("conv1_b", [128], F32, kind="ExternalInput").ap()
    c2w = nc.dram_tensor("conv2_w", [256, 128, 3, 3], F32, kind="ExternalInput").ap()
    c2b = nc.dram_tensor("conv2_b", [256], F32, kind="ExternalInput").ap()
    c3w = nc.dram_tensor("conv3_w", [256, 1, 9, 9], F32, kind="ExternalInput").ap()
    c3b = nc.dram_tensor("conv3_b", [256], F32, kind="ExternalInput").ap()
    bn = {}
    for i, nch in ((1, 128), (2, 256)):
        for p in "gbmv":
            bn[f"{i}{p}"] = nc.dram_tensor(f"bn{i}_{p}", [nch], F32, kind="ExternalInput").ap()
    Ws = [nc.dram_tensor(f"W{i+1}", [32, CAPS[i][0], 8, CAPS[i][1]], F32R,
                         kind="ExternalInput").ap() for i in range(3)]
    out = nc.dram_tensor("out", [B, 10, 16], F32, kind="ExternalOutput").ap()

    with tile.TileContext(nc) as tc:
        import contextlib
        ctx = contextlib.ExitStack()
        with ctx:
            wp = ctx.enter_context(tc.tile_pool(name="wp", bufs=1))
            io = ctx.enter_context(tc.tile_pool(name="io", bufs=1))
            rtp = ctx.enter_context(tc.tile_pool(name="rtp", bufs=2))
            h1p = ctx.enter_context(tc.tile_pool(name="h1p", bufs=3))
            h2p = ctx.enter_context(tc.tile_pool(name="h2p", bufs=3))
            t3p = ctx.enter_context(tc.tile_pool(name="t3p", bufs=3))
            cp = ctx.enter_context(tc.tile_pool(name="cp", bufs=2))
            cq = ctx.enter_context(tc.tile_pool(name="cq", bufs=1))
            ps1 = ctx.enter_context(tc.tile_pool(name="ps1", bufs=2, space="PSUM"))
            ps2 = ctx.enter_context(tc.tile_pool(name="ps2", bufs=3, space="PSUM"))
            psc = ctx.enter_context(tc.tile_pool(name="psc", bufs=3, space="PSUM"))

            # ================= constants =================
            epsn = wp.tile([128, 1], F32, tag="epsn")
            nc.gpsimd.memset(epsn[:], 1e-30)
            epsbn = wp.tile([128, 1], F32, tag="epsbn")
            nc.gpsimd.memset(epsbn[:], BN_EPS)
            ident = wp.tile([128, 128], F32, tag="ident")
            make_identity(nc, ident[:])

            # m4[p, il] = 1 if (p//8)%4 == il  (band select over fake (il, r) grid)
            m4t = wp.tile([128, 4, 4], F32, tag="m4t")
            nc.gpsimd.memset(m4t[:], 0.0)
            nc.gpsimd.affine_select(
                out=m4t[:], in_=m4t[:], compare_op=ALU.is_gt, fill=1.0,
                base=1 - 8, pattern=[[-8, 4], [-32, 4]], channel_multiplier=1)
            nc.gpsimd.affine_select(
                out=m4t[:], in_=m4t[:], compare_op=ALU.is_ge, fill=0.0,
                base=0, pattern=[[-8, 4], [-32, 4]], channel_multiplier=1)
            m4 = wp.tile([128, 4], F32, tag="m4")
            nc.vector.tensor_reduce(m4[:], m4t[:], axis=AX.X, op=ALU.max)

            # R8[t][p, i'] = 1 if p//8 + 16t == i'   (f32r stationary [128, 32])
            R8 = [wp.tile([128, 32], F32R, tag=f"R8_{t}", name=f"R8_{t}") for t in range(2)]
            Rrep = [wp.tile([32, 128], F32R, tag=f"Rrep_{t}", name=f"Rrep_{t}") for t in range(2)]
            for t in range(2):
                v = wp.tile([128, 32], F32, tag="Rscr", name=f"R8s{t}")
                nc.gpsimd.memset(v[:], 0.0)
                nc.gpsimd.affine_select(
                    out=v[:], in_=v[:], compare_op=ALU.is_gt, fill=1.0,
                    base=1 - 8 + 128 * t, pattern=[[-8, 32]], channel_multiplier=1)
                nc.gpsimd.affine_select(
                    out=v[:], in_=v[:], compare_op=ALU.is_ge, fill=0.0,
                    base=128 * t, pattern=[[-8, 32]], channel_multiplier=1)
                nc.vector.tensor_copy(R8[t][:], v[:])
            # Rrep[t][i', q] = 1 if q//8 + 16t == i'  (f32r stationary [32, 128])
            for t in range(2):
                v = wp.tile([32, 128], F32, tag="Rscr2", name=f"Rreps{t}")
                nc.gpsimd.memset(v[:], 0.0)
                nc.gpsimd.affine_select(
                    out=v[:], in_=v[:], compare_op=ALU.is_gt, fill=1.0,
                    base=1 - 8 + 128 * t, pattern=[[1, 128]], channel_multiplier=-8)
                nc.gpsimd.affine_select(
                    out=v[:], in_=v[:], compare_op=ALU.is_ge, fill=0.0,
                    base=128 * t, pattern=[[1, 128]], channel_multiplier=-8)
                nc.vector.tensor_copy(Rrep[t][:], v[:])
            # F[p, b] = 1 if p%32 == b  (bf16 stationary for the strip fold)
            Ff32 = wp.tile([128, 32], F32, tag="Ff32")
            nc.gpsimd.memset(Ff32[:], 0.0)
            for r in range(4):
                nc.gpsimd.affine_select(
                    out=Ff32[:], in_=Ff32[:], compare_op=ALU.not_equal, fill=1.0,
                    base=-32 * r, pattern=[[-1, 32]], channel_multiplier=1)
            Ffold = wp.tile([128, 32], BF16, tag="Ffold")
            nc.scalar.copy(Ffold[:], Ff32[:])

            # ================= BN / bias prep =================
            def bn_prep(nch, g, b_, m, v):
                nh = nch // 128
                gt = wp.tile([128, nh], F32, tag=f"bng{nch}")
                bt = wp.tile([128, nh], F32, tag=f"bnb{nch}")
                mt = wp.tile([128, nh], F32, tag=f"bnm{nch}")
                vt = wp.tile([128, nh], F32, tag=f"bnv{nch}")
                for h in range(nh):
                    nc.sync.dma_start(gt[:, h], g[128 * h:128 * (h + 1)])
                    nc.sync.dma_start(bt[:, h], b_[128 * h:128 * (h + 1)])
                    nc.sync.dma_start(mt[:, h], m[128 * h:128 * (h + 1)])
                    nc.sync.dma_start(vt[:, h], v[128 * h:128 * (h + 1)])
                ln = wp.tile([128, nh], F32, tag=f"bnl{nch}")
                nc.scalar.activation(ln[:], vt[:], ACTF.Ln, bias=epsbn[:, 0:1])
                rsd = wp.tile([128, nh], F32, tag=f"bnr{nch}")
                nc.scalar.activation(rsd[:], ln[:], ACTF.Exp, scale=-0.5)
                inv = wp.tile([128, nh], F32, tag=f"bni{nch}")
                nc.vector.tensor_tensor(inv[:], gt[:], rsd[:], ALU.mult)
                ms = wp.tile([128, nh], F32, tag=f"bnms{nch}")
                nc.vector.tensor_tensor(ms[:], mt[:], inv[:], ALU.mult)
                shift = wp.tile([128, nh], F32, tag=f"bnsh{nch}")
                nc.vector.tensor_tensor(shift[:], bt[:], ms[:], ALU.subtract)
                return inv, shift

            inv1, shift1 = bn_prep(128, bn["1g"], bn["1b"], bn["1m"], bn["1v"])
            inv2, shift2 = bn_prep(256, bn["2g"], bn["2b"], bn["2m"], bn["2v"])
            c1bt = wp.tile([128, 1], F32, tag="c1bt")
            nc.sync.dma_start(c1bt[:, 0], c1b[:])
            t_ = wp.tile([128, 1], F32, tag="c1bs")
            nc.vector.tensor_tensor(t_[:], c1bt[:], inv1[:], ALU.mult)
            nc.vector.tensor_tensor(shift1[:], shift1[:], t_[:], ALU.add)
            c2bt = wp.tile([128, 2], F32, tag="c2bt")
            for h in range(2):
                nc.sync.dma_start(c2bt[:, h], c2b[128 * h:128 * (h + 1)])
            t2_ = wp.tile([128, 2], F32, tag="c2bs")
            nc.vector.tensor_tensor(t2_[:], c2bt[:], inv2[:], ALU.mult)
            nc.vector.tensor_tensor(shift2[:], shift2[:], t2_[:], ALU.add)
            c3bt = wp.tile([128, 2], F32, tag="c3bt")
            for h in range(2):
                nc.sync.dma_start(c3bt[:, h], c3b[128 * h:128 * (h + 1)])

            # ================= conv weights =================
            w1nat = wp.tile([128, 9], F32, tag="w1nat")
            nc.sync.dma_start(w1nat[:], c1w.rearrange("co i kh kw -> co (i kh kw)"))
            w1ps = psc.tile([9, 128], F32, tag="cap", name="w1ps")
            nc.tensor.transpose(w1ps[:], w1nat[:], ident[:])
            w1T = wp.tile([9, 128], BF16, tag="w1T")
            nc.scalar.copy(w1T[:], w1ps[:])

            w2T = wp.tile([128, 18, 128], BF16, tag="w2T")
            for h in range(2):
                w2nat = wp.tile([128, 1152], F32, tag="w2nat", name=f"w2nat{h}")
                nc.gpsimd.dma_start(
                    w2nat[:],
                    c2w.rearrange("co ci kh kw -> co (ci kh kw)")[128 * h:128 * (h + 1), :])
                for p in range(9):
                    wps = psc.tile([128, 128], F32, tag="cap", name=f"w2ps{h}{p}")
                    nc.tensor.transpose(wps[:], w2nat[:, p::9], ident[:])
                    nc.scalar.copy(w2T[:, 2 * p + h, :], wps[:])

            w3f = wp.tile([128, 2, 81], F32, tag="w3f")
            for h in range(2):
                nc.gpsimd.dma_start(
                    w3f[:, h, :],
                    c3w.rearrange("c i kh kw -> c (i kh kw)")[128 * h:128 * (h + 1), :])
            w3t = wp.tile([128, 2, 81], BF16, tag="w3t")
            nc.scalar.copy(w3t[:], w3f[:])

            # ================= capsule weights =================
            dn = []     # dn[L][t]      [128=(r,il,j), KL] f32r  (dense moving)
            dn_il = []  # dn_il[L][t][il]                        (il-masked)
            for L, (NH, DH) in enumerate(CAPS):
                KL = NH * DH
                Wst = wp.tile([32, NH * 8 * DH], F32R, tag="Wst", name=f"Wst{L}")
                nc.gpsimd.dma_start(Wst[:], Ws[L].rearrange("i k j l -> i (k j l)"))
                Wst2 = wp.tile([32, 8, NH, DH], F32R, tag="Wst2", name=f"Wst2{L}")
                nc.vector.tensor_copy(
                    Wst2[:],
                    Wst[:].bitcast(F32).rearrange("i (k j l) -> i j k l", k=NH, j=8))
                dt_, dmt = [], []
                for t in range(2):
                    d = wp.tile([128, KL], F32R, tag=f"dn{L}{t}")
                    nc.gpsimd.dma_start(
                        d[:],
                        Wst2[16 * t:16 * (t + 1), :, :, :].rearrange("p j k l -> p j (k l)"))
                    ils = []
                    for il in range(4):
                        dm = wp.tile([128, KL], F32R, tag=f"dnm{L}{t}{il}")
                        eng = nc.vector if (il % 2 == 0) else nc.gpsimd
                        eng.tensor_tensor(
                            dm[:], d[:].bitcast(F32),
                            m4[:, il:il + 1].broadcast_to([128, KL]), ALU.mult)
                        ils.append(dm)
                    dt_.append(d)
                    dmt.append(ils)
                dn.append(dt_)
                dn_il.append(dmt)

            # ================= input x =================
            xmr = io.tile([B, 1521], F32, tag="xmr")
            nc.sync.dma_start(xmr[:], x.rearrange("b i h w -> b (i h w)"))
            xm = io.tile([B, 39, 41], BF16, tag="xm")
            nc.gpsimd.memset(xm[:, :, 39:41], 0.0)
            nc.scalar.copy(xm[:, :, 0:39], xmr[:].rearrange("b (h w) -> b h w", h=39))

            # h3 accumulators per (sb, t): [128, SB] f32
            h3sb = [[cp.tile([128, SB], F32, tag=f"h3_{s}_{t}", name=f"h3_{s}_{t}") for t in range(2)]
                    for s in range(NSB)]

            uT4 = [[None, None] for _ in range(NSB)]
            uB = [[None, None] for _ in range(NSB)]

            for sb in range(NSB):
                # ======== convs ========
                for blk in range(SB // RBLK):
                    b0 = sb * SB + blk * RBLK
                    # Rt[p=(kh,kw), img, r(19), c(0:39 used, 40 pitch)]
                    #   <- xm[b, kh+2r, kw+c]
                    Rt = rtp.tile([9, RBLK, 19, 40], BF16, tag="Rt")
                    for p in range(9):
                        kh, kw = p // 3, p % 3
                        nc.sync.dma_start(
                            Rt[p:p + 1, :, :, 0:39],
                            xm[b0:b0 + RBLK, kh:kh + 37:2, kw:kw + 39])
                    for ch2 in range(RBLK // BC):
                        chl = blk * (RBLK // BC) + ch2   # chunk in sb: 0..7
                        h1c = h1p.tile([128, BC, 19, 20], BF16, tag="h1c")
                        for i in range(BC):
                            i2 = ch2 * BC + i
                            pst = ps1.tile([128, 19, 20], F32, tag="ps1")
                            nc.tensor.matmul(
                                pst[:], w1T[0:9, :], Rt[:, i2, :, 0:39:2],
                                start=True, stop=True)
                            nc.scalar.activation(
                                h1c[:, i, :, :], pst[:], ACTF.Relu,
                                bias=shift1[:, 0:1], scale=inv1[:, 0:1])
                        for h in range(2):
                            ps2t = ps2.tile([128, BC, 9, 9], F32, tag="ps2")
                            for p in range(9):
                                kh, kw = p // 3, p % 3
                                nc.tensor.matmul(
                                    ps2t[:],
                                    w2T[:, 2 * p + h, :],
                                    h1c[:, :, kh:kh + 17:2, kw:kw + 17:2],
                                    start=(p == 0), stop=(p == 8))
                            h2c = h2p.tile([128, BC, 81], BF16, tag="h2c")
                            nc.scalar.activation(
                                h2c[:], ps2t[:].rearrange("p b i j -> p b (i j)"),
                                ACTF.Relu, bias=shift2[:, h:h + 1], scale=inv2[:, h:h + 1])
                            t3 = t3p.tile([128, BC, 81], F32, tag="t3")
                            nc.gpsimd.tensor_tensor(
                                t3[:], h2c[:],
                                w3t[:, h:h + 1, :].broadcast_to([128, BC, 81]), ALU.mult)
                            nc.vector.tensor_reduce(
                                h3sb[sb][h][:, BC * chl:BC * (chl + 1)], t3[:],
                                axis=AX.X, op=ALU.add)

                # ======== squash(u0) ========
                h3b, sqt = [], []
                for t in range(2):
                    hb = cp.tile([128, SB], F32, tag=f"h3b{sb}{t}")
                    nc.vector.tensor_tensor(
                        hb[:], h3sb[sb][t][:],
                        c3bt[:, t:t + 1].broadcast_to([128, SB]), ALU.add)
                    h3b.append(hb)
                    sq = cp.tile([128, SB], F32R, tag=f"sq{sb}{t}")
                    nc.vector.tensor_tensor(sq[:], hb[:], hb[:], ALU.mult)
                    sqt.append(sq)
                ssps = psc.tile([32, SB], F32, tag="cap", name=f"ss{sb}")
                for t in range(2):
                    nc.tensor.matmul(ssps[:], R8[t][:], sqt[t][:],
                                     start=(t == 0), stop=(t == 1))
                lnss = cq.tile([32, SB], F32, tag=f"lns{sb}")
                nc.scalar.activation(lnss[:], ssps[:], ACTF.Ln, bias=epsn[0:32, 0:1])
                nrm = cq.tile([32, SB], F32, tag=f"nrm{sb}")
                nc.scalar.activation(nrm[:], lnss[:], ACTF.Exp, scale=0.5)
                e0 = cq.tile([32, SB], F32, tag=f"e0{sb}")
                nc.scalar.activation(e0[:], nrm[:], ACTF.Exp, scale=-1.0)
                f0 = cq.tile([32, SB], F32, tag=f"f0{sb}")
                nc.vector.tensor_scalar(f0[:], e0[:], -1.0, 1.0, ALU.mult, ALU.add)
                rn = cq.tile([32, SB], F32, tag=f"rn{sb}")
                nc.vector.reciprocal(rn[:], nrm[:])
                g0 = cp.tile([32, SB], F32R, tag=f"g0{sb}")
                nc.vector.tensor_tensor(g0[:], f0[:], rn[:], ALU.mult)
                for t in range(2):
                    grps = psc.tile([128, SB], F32, tag="cap", name=f"gr{sb}{t}")
                    nc.tensor.matmul(grps[:], Rrep[t][:], g0[:], start=True, stop=True)
                    u0 = cp.tile([128, SB], F32, tag=f"u0{sb}{t}")
                    nc.vector.tensor_tensor(u0[:], h3b[t][:], grps[:], ALU.mult)
                    u4 = cp.tile([128, 4, SB], F32R, tag=f"uT4{sb}{t}")
                    nc.vector.tensor_copy(
                        u4[:], u0[:, None, :].broadcast_to([128, 4, SB]))
                    uT4[sb][t] = u4
                    ub = cp.tile([128, 128], F32R, tag=f"uB{sb}{t}")
                    nc.gpsimd.memset(ub[:].bitcast(F32), 0.0)
                    for r in range(4):
                        nc.gpsimd.tensor_copy(
                            ub[32 * r:32 * r + 32, 32 * r:32 * r + 32],
                            u0[32 * r:32 * r + 32, :])
                    uB[sb][t] = ub

                # ======== capsule layers ========
                for L, (NH, DH) in enumerate(CAPS):
                    KL = NH * DH
                    # S (replicated over r) via uT4 stationary
                    Sps = psc.tile([128, KL], F32, tag="cap", name=f"S{L}{sb}")
                    for t in range(2):
                        nc.tensor.matmul(
                            Sps[:],
                            uT4[sb][t][:].rearrange("p r b -> p (r b)"),
                            dn[L][t][:], start=(t == 0), stop=(t == 1))
                    Srep = cq.tile([128, KL], F32, tag=f"Sr{sb}")
                    nc.vector.tensor_copy(Srep[:], Sps[:])
                    rS = cq.tile([32, KL], F32, tag=f"rS{sb}")
                    nc.vector.reciprocal(rS[:], Srep[0:32, :])
                    # T strips + A
                    A = cq.tile([128, 8, NH], F32, tag=f"A{sb}")
                    q2s = {}
                    for t in range(2):
                        for pr in range(2):
                            Tps = psc.tile([128, 2, KL], F32, tag="cap",
                                           name=f"T{L}{sb}{t}{pr}")
                            for i2 in range(2):
                                nc.tensor.matmul(
                                    Tps[:, i2, :], uB[sb][t][:],
                                    dn_il[L][t][2 * pr + i2][:],
                                    start=True, stop=True)
                            q2 = cq.tile([128, 2, KL], BF16, tag=f"q2_{sb}{t}{pr}")
                            nc.vector.tensor_tensor(
                                q2[:], Tps[:],
                                Srep[:, None, :].broadcast_to([128, 2, KL]), ALU.mult)
                            q2s[(t, pr)] = q2
                            q = 2 * t + pr
                            nc.vector.tensor_reduce(
                                A[:, 2 * q:2 * q + 2, :],
                                q2[:].rearrange("p i (k l) -> p i k l", k=NH),
                                axis=AX.X, op=ALU.add)
                    # softmax over k
                    Mx = cq.tile([128, 8], F32, tag=f"Mx{sb}")
                    nc.vector.tensor_reduce(Mx[:], A[:], axis=AX.X, op=ALU.max)
                    Ms = cq.tile([128, 8], F32, tag=f"Ms{sb}")
                    nc.scalar.mul(Ms[:], Mx[:], ISCALE)
                    E = cq.tile([128, 8, NH], F32, tag=f"E{sb}")
                    nc.vector.scalar_tensor_tensor(
                        E[:], A[:], ISCALE,
                        Ms[:, :, None].broadcast_to([128, 8, NH]),
                        ALU.mult, ALU.subtract)
                    nc.scalar.activation(E[:], E[:], ACTF.Exp)
                    Z = cq.tile([128, 8], F32, tag=f"Z{sb}")
                    nc.vector.tensor_reduce(Z[:], E[:], axis=AX.X, op=ALU.add)
                    rZ = cq.tile([128, 8], F32, tag=f"rZ{sb}")
                    nc.vector.reciprocal(rZ[:], Z[:])
                    C = cq.tile([128, 8, NH], BF16, tag=f"C{sb}")
                    nc.vector.tensor_tensor(
                        C[:], E[:], rZ[:, :, None].broadcast_to([128, 8, NH]), ALU.mult)
                    # weighted strips -> fold into Uh psum
                    Uhps = psc.tile([32, KL], F32, tag="cap", name=f"Uh{L}{sb}")
                    first = True
                    for t in range(2):
                        for pr in range(2):
                            q = 2 * t + pr
                            q3 = cq.tile([128, 2, KL], BF16, tag=f"q3_{sb}{t}{pr}")
                            eng = nc.vector if pr == 0 else nc.gpsimd
                            eng.tensor_tensor(
                                q3[:].rearrange("p i (k l) -> p i k l", k=NH),
                                q2s[(t, pr)][:].rearrange("p i (k l) -> p i k l", k=NH),
                                C[:, 2 * q:2 * q + 2, :, None]
                                .broadcast_to([128, 2, NH, DH]), ALU.mult)
                            for i2 in range(2):
                                nc.tensor.matmul(
                                    Uhps[:], Ffold[:], q3[:, i2, :],
                                    start=first, stop=(t == 1 and pr == 1 and i2 == 1))
                                first = False
                    # undo the S factor baked into q2 (= T*S), then squash
                    Uh2 = cq.tile([32, KL], F32, tag=f"Uh2{sb}")
                    nc.vector.tensor_tensor(Uh2[:], Uhps[:], rS[:], ALU.mult)
                    usq = cq.tile([32, KL], F32, tag=f"usq{sb}")
                    nc.scalar.activation(usq[:], Uh2[:], ACTF.Square)
                    ss2 = cq.tile([32, NH], F32, tag=f"ss2{sb}")
                    nc.vector.tensor_reduce(
                        ss2[:], usq[:].rearrange("b (k l) -> b k l", k=NH),
                        axis=AX.X, op=ALU.add)
                    ln2 = cq.tile([32, NH], F32, tag=f"ln2{sb}")
                    nc.scalar.activation(ln2[:], ss2[:], ACTF.Ln, bias=epsn[0:32, 0:1])
                    n2 = cq.tile([32, NH], F32, tag=f"n2{sb}")
                    nc.scalar.activation(n2[:], ln2[:], ACTF.Exp, scale=0.5)
                    e2 = cq.tile([32, NH], F32, tag=f"e2{sb}")
                    nc.scalar.activation(e2[:], n2[:], ACTF.Exp, scale=-1.0)
                    f2 = cq.tile([32, NH], F32, tag=f"f2{sb}")
                    nc.vector.tensor_scalar(f2[:], e2[:], -1.0, 1.0, ALU.mult, ALU.add)
                    rn2 = cq.tile([32, NH], F32, tag=f"rn2{sb}")
                    nc.vector.reciprocal(rn2[:], n2[:])
                    g2 = cq.tile([32, NH], F32, tag=f"g2{sb}")
                    nc.vector.tensor_tensor(g2[:], f2[:], rn2[:], ALU.mult)
                    un = cq.tile([32, KL], F32, tag=f"un{sb}")
                    nc.vector.tensor_tensor(
                        un[:].rearrange("b (k l) -> b k l", k=NH),
                        Uh2[:].rearrange("b (k l) -> b k l", k=NH),
                        g2[:, :, None].broadcast_to([32, NH, DH]), ALU.mult)
                    if L < 2:
                        for t in range(2):
                            uTf = cp.tile([128, SB], F32, tag=f"uTf{sb}{t}")
                            for c in range(4):
                                nc.vector.transpose(
                                    uTf[32 * c:32 * c + 32, :],
                                    un[:, 128 * t + 32 * c:128 * t + 32 * c + 32])
                            u4 = cp.tile([128, 4, SB], F32R, tag=f"uT4{sb}{t}")
                            nc.vector.tensor_copy(
                                u4[:],
                                uTf[:, None, :].broadcast_to([128, 4, SB]))
                            uT4[sb][t] = u4
                            ub = cp.tile([128, 128], F32R, tag=f"uB{sb}{t}")
                            nc.gpsimd.memset(ub[:].bitcast(F32), 0.0)
                            for r in range(4):
                                nc.gpsimd.tensor_copy(
                                    ub[32 * r:32 * r + 32, 32 * r:32 * r + 32],
                                    uTf[32 * r:32 * r + 32, :])
                            uB[sb][t] = ub
                    else:
                        nc.sync.dma_start(
                            out[SB * sb:SB * (sb + 1), :, :],
                            un[:].rearrange("b (k l) -> b k l", k=NH))

    nc.compile()
    return nc


_NC_CACHE = {}


def _get_nc(debug=False):
    if debug not in _NC_CACHE:
        _NC_CACHE[debug] = build()
    return _NC_CACHE[debug]


def kernel(**inputs):
    nc = _get_nc(False)
    x = np.ascontiguousarray(inputs["x"], dtype=np.float32)
    names = ["conv1_w", "conv1_b", "conv2_w", "conv2_b", "conv3_w", "conv3_b",
             "W1", "W2", "W3"]
    base = {n: np.ascontiguousarray(inputs[n], dtype=np.float32) for n in names}
    for i in (1, 2):
        for p in "gbmv":
            base[f"bn{i}_{p}"] = np.ascontiguousarray(inputs[f"bn{i}_{p}"], dtype=np.float32)
    in_maps = []
    for c in range(8):
        m = dict(base)
        m["x"] = x[B * c:B * (c + 1)]
        in_maps.append(m)
    res = run_bass_kernel_spmd(nc, in_maps, core_ids=list(range(8)))
    return np.concatenate([res.results[i]["out"] for i in range(8)], axis=0)
